# revision 1
# baseline (speedup 1.0000x reference)
import numpy as np

# Single causal self-attention head: x [512,256,384], Wk/Wq/Wv [384,64].
# Data parallel: shard B=512 across 8 NeuronCores (64 per core), weights replicated.

B, T, C, H, M = 512, 256, 384, 64, 8


def _attn_np(x, Wk, Wq, Wv):
    k = x @ Wk
    q = x @ Wq
    v = x @ Wv
    wei = np.einsum('bth,bsh->bts', q, k) * (1.0 / np.sqrt(H))
    mask = np.tril(np.ones((T, T), dtype=bool))
    wei = np.where(mask, wei, -np.inf)
    wei = wei - wei.max(axis=-1, keepdims=True)
    e = np.exp(wei)
    wei = e / e.sum(axis=-1, keepdims=True)
    return np.einsum('bts,bsh->bth', wei, v).astype(np.float32)


def kernel(x, Wk, Wq, Wv):
    x = np.asarray(x, np.float32)
    Wk = np.asarray(Wk, np.float32)
    Wq = np.asarray(Wq, np.float32)
    Wv = np.asarray(Wv, np.float32)
    try:
        import jax
        import jax.numpy as jnp

        devs = jax.devices()[:M]
        if len(devs) < M:
            raise RuntimeError("need 8 cores")

        def head(xs, wk, wq, wv):
            k = jnp.einsum('btc,ch->bth', xs, wk)
            q = jnp.einsum('btc,ch->bth', xs, wq)
            v = jnp.einsum('btc,ch->bth', xs, wv)
            wei = jnp.einsum('bth,bsh->bts', q, k) * (1.0 / np.sqrt(H))
            causal = jnp.tril(jnp.ones((T, T), dtype=bool))
            wei = jnp.where(causal, wei, -jnp.inf)
            wei = jax.nn.softmax(wei, axis=-1)
            return jnp.einsum('bts,bsh->bth', wei, v)

        pm = jax.pmap(head, devices=devs)
        xs = x.reshape(M, B // M, T, C)
        wk = np.broadcast_to(Wk, (M,) + Wk.shape)
        wq = np.broadcast_to(Wq, (M,) + Wq.shape)
        wv = np.broadcast_to(Wv, (M,) + Wv.shape)
        out = np.asarray(pm(xs, wk, wq, wv))
        return out.reshape(B, T, H).astype(np.float32)
    except Exception:
        return _attn_np(x, Wk, Wq, Wv)
